# revision 1
# baseline (speedup 1.0000x reference)
"""Trainium2 Bass kernel for a DeepSeek-style MoE block (full-I/O contract).

Strategy (8 NeuronCores):
  - Expert-parallel: E=16 routed experts, 2 per core. Host computes the gate
    (softmax + top-4) in numpy, gathers each expert's tokens, and ships
    transposed token blocks per core. Experts are ranked by token count:
    ranks 0-7 go to slot 0 (capacity C0), ranks 8-15 to slot 1 (C1 <= C0),
    so padding waste tracks the actual load distribution.
  - Shared expert is tensor-parallel along its intermediate dim Fs=2816:
    each core owns a 352-wide slice (zero-padded to 384 = 3*128).
  - All matmuls run in bf16 with fp32 PSUM accumulation; silu on the ACT
    engine in fp32; routing weights applied per-token at PSUM eviction.
  - Host scatter-adds the routed partials and sums the shared partials.
"""

import math
from contextlib import ExitStack

import ml_dtypes
import numpy as np

T = 2048
H = 2048
E = 16
TOP_K = 4
F = 1408
FS = 2816
N_CORES = 8
EPC = E // N_CORES  # experts per core = 2
KH = H // 128  # 16 contraction chunks over H
NF = F // 128  # 11 F tiles
NH = H // 512  # 4 output H tiles
FSS = FS // N_CORES  # 352 shared slice per core
FSP = 384  # padded to 3*128
NFS = FSP // 128  # 3
NT = T // 512  # 4

BF16 = ml_dtypes.bfloat16

_BUILD_CACHE: dict[tuple, object] = {}
last_exec_time_ns = None


def _routing(x: np.ndarray, gate_weight: np.ndarray):
    """Replicates the reference gate: fp32 logits, softmax, top-4 (ties ->
    lower expert index, matching jax.lax.top_k), no renorm."""
    logits = x.astype(np.float32) @ gate_weight.astype(np.float32).T
    z = logits - logits.max(axis=1, keepdims=True)
    p = np.exp(z)
    p /= p.sum(axis=1, keepdims=True)
    top_idx = np.argsort(-p, axis=1, kind="stable")[:, :TOP_K]
    top_vals = np.take_along_axis(p, top_idx, axis=1).astype(np.float32)
    return top_idx, top_vals


def _chunks(C):
    n = max(1, math.ceil(C / 512))
    while C % n:
        n += 1
    return C // n


def _build(caps: tuple):
    """Build + compile the SPMD one-core Bass graph for slot capacities."""
    key = tuple(caps)
    if key in _BUILD_CACHE:
        return _BUILD_CACHE[key]

    import concourse.bass as bass  # noqa: F401
    from concourse import bacc, mybir, tile

    bf = mybir.dt.bfloat16
    f32 = mybir.dt.float32
    Silu = mybir.ActivationFunctionType.Silu

    CTs = [(C + 127) // 128 for C in caps]
    CTsum = sum(CTs)
    off = [0, caps[0]]  # row offsets into rout

    nc = bacc.Bacc(None, target_bir_lowering=False)

    xg_ds = [
        nc.dram_tensor(f"xg{e}", [128, KH, caps[e]], bf, kind="ExternalInput")
        for e in range(EPC)
    ]
    wg_d = nc.dram_tensor("wg", [EPC, NF, 128, KH, 128], bf, kind="ExternalInput")
    wu_d = nc.dram_tensor("wu", [EPC, NF, 128, KH, 128], bf, kind="ExternalInput")
    wd_d = nc.dram_tensor("wd", [EPC, NH, 128, NF, 512], bf, kind="ExternalInput")
    xt_d = nc.dram_tensor("xt", [NT, 128, KH, 512], bf, kind="ExternalInput")
    wsg_d = nc.dram_tensor("wsg", [NFS, 128, KH, 128], bf, kind="ExternalInput")
    wsu_d = nc.dram_tensor("wsu", [NFS, 128, KH, 128], bf, kind="ExternalInput")
    wsd_d = nc.dram_tensor("wsd", [NH, 128, NFS, 512], bf, kind="ExternalInput")
    wts_d = nc.dram_tensor("wts", [128, CTsum], f32, kind="ExternalInput")
    rout = nc.dram_tensor("rout", [sum(caps), H], bf, kind="ExternalOutput")
    sout = nc.dram_tensor("sout", [T, H], bf, kind="ExternalOutput")

    with tile.TileContext(nc) as tc, ExitStack() as ctx:
        const = ctx.enter_context(tc.tile_pool(name="const", bufs=1))
        bias0 = const.tile([128, 1], f32)
        nc.vector.memset(bias0[:], 0.0)
        wts_t = const.tile([128, CTsum], f32)
        nc.sync.dma_start(wts_t[:], wts_d[:])


        xt_pool = ctx.enter_context(tc.tile_pool(name="xtp", bufs=2))
        wgu_pool = ctx.enter_context(tc.tile_pool(name="wgu", bufs=4))
        as_pool = ctx.enter_context(tc.tile_pool(name="asp", bufs=2))
        wsd_pool = ctx.enter_context(tc.tile_pool(name="wsdp", bufs=1))
        ev_pool = ctx.enter_context(tc.tile_pool(name="evp", bufs=6))
        sg_pool = ctx.enter_context(tc.tile_pool(name="sgp", bufs=4))
        psum_gu = ctx.enter_context(tc.tile_pool(name="pgu", bufs=2, space="PSUM"))
        psum_d = ctx.enter_context(tc.tile_pool(name="pdp", bufs=4, space="PSUM"))

        # HAM warmup: keep the PE busy during the initial DMA wait so the
        # clock-gate is at 8/8 when the first real matmuls arrive
        warm = const.tile([128, 512], bf, name="warm")
        nc.vector.memset(warm[:], 0.0)
        warmout = const.tile([128, 1], f32, name="warmout")
        wpsum = psum_gu.tile([128, 512], f32, tag="pg", name="warmp")
        for _ in range(30):
            nc.tensor.matmul(wpsum, warm[:, :128], warm[:], start=True, stop=True)
        nc.vector.tensor_copy(out=warmout[:], in_=wpsum[:, :1])

        # ---- shared expert (Fs tensor-parallel slice) ----
        # down-proj weight tiles (12 KB/partition) load after the first
        # gate/up chunk is emitted, keeping the critical-path DMAs in front
        wsd_ts = []
        for j in range(NT):
            tsl = slice(j * 512, (j + 1) * 512)
            xt_t = xt_pool.tile([128, KH, 512], bf, tag="xt", name="xt")
            if j == 0:
                for q in range(4):
                    nc.sync.dma_start(
                        xt_t[:, q * 4:(q + 1) * 4], xt_d[j, :, q * 4:(q + 1) * 4]
                    )
            else:
                nc.sync.dma_start(xt_t[:], xt_d[j])
            as_j = as_pool.tile([128, NFS, 512], bf, tag="asj", name="asj")
            for f in range(NFS):
                wsg_t = wgu_pool.tile([128, KH, 128], bf, tag="wg")
                wsu_t = wgu_pool.tile([128, KH, 128], bf, tag="wu")
                if j == 0 and f == 0:
                    nc.sync.dma_start(wsg_t[:, :8], wsg_d[f, :, :8])
                    nc.sync.dma_start(wsg_t[:, 8:], wsg_d[f, :, 8:])
                    nc.sync.dma_start(wsu_t[:, :8], wsu_d[f, :, :8])
                    nc.sync.dma_start(wsu_t[:, 8:], wsu_d[f, :, 8:])
                else:
                    nc.sync.dma_start(wsg_t[:], wsg_d[f])
                    nc.sync.dma_start(wsu_t[:], wsu_d[f])
                pg = psum_gu.tile([128, 512], f32, tag="pg", name="pg")
                pu = psum_gu.tile([128, 512], f32, tag="pu", name="pu")
                for k in range(KH):
                    nc.tensor.matmul(
                        pg, wsg_t[:, k], xt_t[:, k],
                        start=(k == 0), stop=(k == KH - 1),
                    )
                for k in range(KH):
                    nc.tensor.matmul(
                        pu, wsu_t[:, k], xt_t[:, k],
                        start=(k == 0), stop=(k == KH - 1),
                    )
                sg = sg_pool.tile([128, 512], f32, tag="sg", name="sg")
                nc.scalar.activation(sg, pg, Silu, bias=bias0[:])
                nc.vector.tensor_mul(as_j[:, f], sg, pu)
            if not wsd_ts:
                for h in range(NH):
                    w = wsd_pool.tile(
                        [128, NFS, 512], bf, tag=f"wsd{h}", name=f"wsd{h}"
                    )
                    nc.sync.dma_start(w[:], wsd_d[h])
                    wsd_ts.append(w)
            # down-proj for this T-chunk right away: spreads sout writes
            for tl in range(4):
                for h in range(NH):
                    pd = psum_d.tile([128, 512], f32, tag="pd")
                    for fo in range(NFS):
                        nc.tensor.matmul(
                            pd, as_j[:, fo, tl * 128:(tl + 1) * 128],
                            wsd_ts[h][:, fo],
                            start=(fo == 0), stop=(fo == NFS - 1),
                        )
                    ob = ev_pool.tile([128, 512], bf, tag="ob")
                    nc.any.tensor_copy(out=ob[:], in_=pd[:])
                    nc.sync.dma_start(
                        sout[j * 512 + tl * 128: j * 512 + (tl + 1) * 128,
                             h * 512:(h + 1) * 512],
                        ob[:],
                    )

        # ---- routed experts (2 per core, slot capacities C0 >= C1) ----
        with (
            tc.tile_pool(name="xgp", bufs=1) as xg_pool,
            tc.tile_pool(name="wdp", bufs=2) as wd_pool,
            tc.tile_pool(name="atp", bufs=1) as a_pool,
        ):
            for e in range(EPC):
                C = caps[e]
                CT = CTs[e]
                cw = _chunks(C)
                nch = C // cw
                xg_t = xg_pool.tile([128, KH, C], bf, tag=f"xg{e}", name=f"xg{e}")
                nc.sync.dma_start(xg_t[:], xg_ds[e][:])
                aT = a_pool.tile([128, NF, C], bf, tag=f"aT{e}", name=f"aT{e}")
                for f in range(NF):
                    wg_t = wgu_pool.tile([128, KH, 128], bf, tag="wg")
                    nc.sync.dma_start(wg_t[:], wg_d[e, f])
                    wu_t = wgu_pool.tile([128, KH, 128], bf, tag="wu")
                    nc.sync.dma_start(wu_t[:], wu_d[e, f])
                    for j in range(nch):
                        csl = slice(j * cw, (j + 1) * cw)
                        pg = psum_gu.tile([128, 512], f32, tag="pg", name="pg")[:, :cw]
                        pu = psum_gu.tile([128, 512], f32, tag="pu", name="pu")[:, :cw]
                        for k in range(KH):
                            nc.tensor.matmul(
                                pg, wg_t[:, k], xg_t[:, k, csl],
                                start=(k == 0), stop=(k == KH - 1),
                            )
                        for k in range(KH):
                            nc.tensor.matmul(
                                pu, wu_t[:, k], xg_t[:, k, csl],
                                start=(k == 0), stop=(k == KH - 1),
                            )
                        sg = sg_pool.tile([128, 512], f32, tag="sg", name="sg")[:, :cw]
                        nc.scalar.activation(sg, pg, Silu, bias=bias0[:])
                        nc.vector.tensor_mul(aT[:, f, csl], sg, pu)
                for h in range(NH):
                    wd_t = wd_pool.tile([128, NF, 512], bf, tag="wd")
                    nc.sync.dma_start(wd_t[:], wd_d[e, h])
                    for ct in range(CT):
                        tw = min(128, C - ct * 128)
                        pd = psum_d.tile([128, 512], f32, tag="pd", name="pd")[:tw]
                        for fo in range(NF):
                            nc.tensor.matmul(
                                pd, aT[:, fo, ct * 128: ct * 128 + tw],
                                wd_t[:, fo],
                                start=(fo == 0), stop=(fo == NF - 1),
                            )
                        ob = ev_pool.tile([128, 512], bf, tag="ob", name="ob")[:tw]
                        col = sum(CTs[:e]) + ct
                        nc.vector.tensor_scalar_mul(
                            ob[:], pd[:], wts_t[:tw, col:col + 1]
                        )
                        nc.sync.dma_start(
                            rout[off[e] + ct * 128: off[e] + ct * 128 + tw,
                                 h * 512:(h + 1) * 512],
                            ob[:],
                        )

    nc.compile()
    _BUILD_CACHE[key] = nc
    return nc


def kernel(**inputs: np.ndarray) -> np.ndarray:
    global last_exec_time_ns
    from concourse.bass_utils import run_bass_kernel_spmd

    hs = inputs["hidden_states"]
    x = np.ascontiguousarray(hs.reshape(T, H), dtype=np.float32)
    top_idx, top_vals = _routing(x, inputs["gate_weight"])

    # per-expert token lists (ascending token order)
    rows_per_e = []
    for e in range(E):
        rows, kpos = np.nonzero(top_idx == e)
        rows_per_e.append((rows, top_vals[rows, kpos]))
    counts = np.array([len(r) for r, _ in rows_per_e])
    # rank experts by load: ranks 0..7 -> slot 0 of cores 0..7 (big slots),
    # ranks 8..15 -> slot 1 of cores 7..0 (small slots)
    order = np.argsort(-counts, kind="stable")
    slot_expert = np.empty((N_CORES, EPC), np.int64)
    for i in range(N_CORES):
        slot_expert[i, 0] = order[i]
        slot_expert[i, 1] = order[E - 1 - i]
    cap = lambda n: max(128, ((n + 63) // 64) * 64)
    caps = (
        cap(int(counts[slot_expert[:, 0]].max())),
        cap(int(counts[slot_expert[:, 1]].max())),
    )
    CTs = [(C + 127) // 128 for C in caps]
    CTsum = sum(CTs)

    nc = _build(caps)

    xb = x.astype(BF16)
    # xt chunks [NT, 128, KH, 512]: xt[j, p, k, t'] = x[j*512+t', k*128+p]
    xtR = np.ascontiguousarray(xb.reshape(NT, 512, KH, 128).transpose(0, 3, 2, 1))

    w_gate = inputs["w_gate"]
    w_up = inputs["w_up"]
    w_down = inputs["w_down"]
    ws_gate = inputs["ws_gate"].astype(BF16)
    ws_up = inputs["ws_up"].astype(BF16)
    ws_down = inputs["ws_down"].astype(BF16)

    in_maps = []
    for c in range(N_CORES):
        wtsR = np.zeros((128, CTsum), np.float32)
        wgR = np.empty((EPC, NF, 128, KH, 128), BF16)
        wuR = np.empty((EPC, NF, 128, KH, 128), BF16)
        wdR = np.empty((EPC, NH, 128, NF, 512), BF16)
        imap = {}
        for el in range(EPC):
            C = caps[el]
            CT = CTs[el]
            ge = int(slot_expert[c, el])
            rows, wts = rows_per_e[ge]
            n = len(rows)
            xgR = np.zeros((128, KH, C), BF16)
            if n:
                # [n, H] -> [128, KH, n]
                xgR[:, :, :n] = xb[rows].reshape(n, KH, 128).transpose(2, 1, 0)
                wcol = np.zeros(CT * 128, np.float32)
                wcol[:n] = wts
                base = sum(CTs[:el])
                wtsR[:, base:base + CT] = wcol.reshape(CT, 128).T
            imap[f"xg{el}"] = xgR
            wgR[el] = (
                w_gate[ge].astype(BF16).reshape(KH, 128, NF, 128).transpose(2, 1, 0, 3)
            )
            wuR[el] = (
                w_up[ge].astype(BF16).reshape(KH, 128, NF, 128).transpose(2, 1, 0, 3)
            )
            wdR[el] = (
                w_down[ge].astype(BF16).reshape(NF, 128, NH, 512).transpose(2, 1, 0, 3)
            )
        sl = slice(c * FSS, (c + 1) * FSS)
        wsgp = np.zeros((H, FSP), BF16)
        wsgp[:, :FSS] = ws_gate[:, sl]
        wsup = np.zeros((H, FSP), BF16)
        wsup[:, :FSS] = ws_up[:, sl]
        wsdp = np.zeros((FSP, H), BF16)
        wsdp[:FSS] = ws_down[sl]
        imap.update(
            wg=wgR,
            wu=wuR,
            wd=wdR,
            xt=xtR,
            wsg=np.ascontiguousarray(
                wsgp.reshape(KH, 128, NFS, 128).transpose(2, 1, 0, 3)
            ),
            wsu=np.ascontiguousarray(
                wsup.reshape(KH, 128, NFS, 128).transpose(2, 1, 0, 3)
            ),
            wsd=np.ascontiguousarray(
                wsdp.reshape(NFS, 128, NH, 512).transpose(2, 1, 0, 3)
            ),
            wts=wtsR,
        )
        in_maps.append(imap)

    res = run_bass_kernel_spmd(nc, in_maps, core_ids=list(range(N_CORES)))
    last_exec_time_ns = res.exec_time_ns

    out = np.zeros((T, H), np.float32)
    off = [0, caps[0]]
    for c in range(N_CORES):
        r = res.results[c]
        out += r["sout"].astype(np.float32)
        for el in range(EPC):
            rows, _ = rows_per_e[int(slot_expert[c, el])]
            n = len(rows)
            if n:
                # rows are unique within one expert, so fancy-index add is safe
                out[rows] += r["rout"][off[el]: off[el] + n].astype(np.float32)
    return out.reshape(hs.shape).astype(hs.dtype)



# revision 2
# speedup vs baseline: 1.6332x; 1.6332x over previous
"""Trainium2 Bass kernel for a DeepSeek-style MoE block (full-I/O contract).

Strategy (8 NeuronCores):
  - Expert-parallel: E=16 routed experts, 2 per core. Host computes the gate
    (softmax + top-4) in numpy, gathers each expert's tokens, and ships
    transposed token blocks per core. Experts are ranked by token count:
    ranks 0-7 go to slot 0 (capacity C0), ranks 8-15 to slot 1 (C1 <= C0),
    so padding waste tracks the actual load distribution.
  - Routed experts run in fp8 (e4m3) with DoubleRow matmuls (2x PE
    throughput): weights are host-scaled into e4m3 range (w_gate x32,
    w_up x8, w_down x64), activations quantized on the fly; the silu
    applies 1/32 as its input scale and the routing weights fold the
    remaining 1/(8*64) at PSUM eviction. Accumulation stays fp32.
  - Shared expert is tensor-parallel along its intermediate dim Fs=2816
    and stays in bf16 (it dominates the output norm, so fp8 there would
    blow the error budget).
  - Host scatter-adds the routed partials and sums the shared partials.
"""

import math
from contextlib import ExitStack

import ml_dtypes
import numpy as np

T = 2048
H = 2048
E = 16
TOP_K = 4
F = 1408
FS = 2816
N_CORES = 8
EPC = E // N_CORES  # experts per core = 2
KH = H // 128  # 16 contraction chunks over H
KH2 = KH // 2  # 8 DoubleRow pairs
NF = F // 128  # 11 F tiles
NF2 = NF // 2  # 5 DoubleRow pairs (+1 single tail chunk)
NH = H // 512  # 4 output H tiles
FSS = FS // N_CORES  # 352 shared slice per core
FSP = 384  # padded to 3*128
NFS = FSP // 128  # 3
NT = T // 512  # 4

BF16 = ml_dtypes.bfloat16
E4 = ml_dtypes.float8_e4m3

SWG = 32.0  # w_gate fp8 scale
SWU = 8.0   # w_up fp8 scale (also the a_q scale; keeps |a_q| < 100 << 240)
SD = 64.0   # w_down fp8 scale
WTS_DIV = SWU * SD  # folded into routing weights at eviction

_BUILD_CACHE: dict[tuple, object] = {}
last_exec_time_ns = None


def _routing(x: np.ndarray, gate_weight: np.ndarray):
    """Replicates the reference gate: fp32 logits, softmax, top-4 (ties ->
    lower expert index, matching jax.lax.top_k), no renorm."""
    logits = x.astype(np.float32) @ gate_weight.astype(np.float32).T
    z = logits - logits.max(axis=1, keepdims=True)
    p = np.exp(z)
    p /= p.sum(axis=1, keepdims=True)
    top_idx = np.argsort(-p, axis=1, kind="stable")[:, :TOP_K]
    top_vals = np.take_along_axis(p, top_idx, axis=1).astype(np.float32)
    return top_idx, top_vals


def _chunks(C):
    n = max(1, math.ceil(C / 512))
    while C % n:
        n += 1
    return C // n


def _build(caps: tuple):
    """Build + compile the SPMD one-core Bass graph for slot capacities."""
    key = tuple(caps)
    if key in _BUILD_CACHE:
        return _BUILD_CACHE[key]

    import concourse.bass as bass  # noqa: F401
    from concourse import bacc, mybir, tile

    bf = mybir.dt.bfloat16
    f32 = mybir.dt.float32
    fp8 = mybir.dt.float8e4
    DR = mybir.MatmulPerfMode.DoubleRow
    Silu = mybir.ActivationFunctionType.Silu

    CTs = [(C + 127) // 128 for C in caps]
    CTsum = sum(CTs)
    off = [0, caps[0]]  # row offsets into rout

    nc = bacc.Bacc(None, target_bir_lowering=False)

    xg_ds = [
        nc.dram_tensor(f"xg{e}", [128, KH, caps[e]], fp8, kind="ExternalInput")
        for e in range(EPC)
    ]
    wg_d = nc.dram_tensor("wg", [EPC, NF, 128, KH, 128], fp8, kind="ExternalInput")
    wu_d = nc.dram_tensor("wu", [EPC, NF, 128, KH, 128], fp8, kind="ExternalInput")
    wd_d = nc.dram_tensor("wd", [EPC, NH, 128, NF, 512], fp8, kind="ExternalInput")
    xt_d = nc.dram_tensor("xt", [NT, 128, KH, 512], bf, kind="ExternalInput")
    wsg_d = nc.dram_tensor("wsg", [NFS, 128, KH, 128], bf, kind="ExternalInput")
    wsu_d = nc.dram_tensor("wsu", [NFS, 128, KH, 128], bf, kind="ExternalInput")
    wsd_d = nc.dram_tensor("wsd", [NH, 128, NFS, 512], bf, kind="ExternalInput")
    wts_d = nc.dram_tensor("wts", [128, CTsum], f32, kind="ExternalInput")
    rout = nc.dram_tensor("rout", [sum(caps), H], bf, kind="ExternalOutput")
    sout = nc.dram_tensor("sout", [T, H], bf, kind="ExternalOutput")

    with tile.TileContext(nc) as tc, ExitStack() as ctx:
        const = ctx.enter_context(tc.tile_pool(name="const", bufs=1))
        bias0 = const.tile([128, 1], f32)
        nc.vector.memset(bias0[:], 0.0)
        wts_t = const.tile([128, CTsum], f32)
        nc.sync.dma_start(wts_t[:], wts_d[:])


        xt_pool = ctx.enter_context(tc.tile_pool(name="xtp", bufs=2))
        wgu_pool = ctx.enter_context(tc.tile_pool(name="wgu", bufs=4))
        as_pool = ctx.enter_context(tc.tile_pool(name="asp", bufs=2))
        wsd_pool = ctx.enter_context(tc.tile_pool(name="wsdp", bufs=1))
        ev_pool = ctx.enter_context(tc.tile_pool(name="evp", bufs=6))
        sg_pool = ctx.enter_context(tc.tile_pool(name="sgp", bufs=4))
        psum_gu = ctx.enter_context(tc.tile_pool(name="pgu", bufs=2, space="PSUM"))
        psum_d = ctx.enter_context(tc.tile_pool(name="pdp", bufs=4, space="PSUM"))

        # HAM warmup: keep the PE busy during the initial DMA wait so the
        # clock-gate is at 8/8 when the first real matmuls arrive
        warm = const.tile([128, 512], bf, name="warm")
        nc.vector.memset(warm[:], 0.0)
        warmout = const.tile([128, 1], f32, name="warmout")
        wpsum = psum_gu.tile([128, 512], f32, tag="pg", name="warmp")
        for _ in range(30):
            nc.tensor.matmul(wpsum, warm[:, :128], warm[:], start=True, stop=True)
        nc.vector.tensor_copy(out=warmout[:], in_=wpsum[:, :1])

        # ---- shared expert (Fs tensor-parallel slice, bf16) ----
        # down-proj weight tiles (12 KB/partition) load after the first
        # gate/up chunk is emitted, keeping the critical-path DMAs in front
        wsd_ts = []
        for j in range(NT):
            tsl = slice(j * 512, (j + 1) * 512)
            xt_t = xt_pool.tile([128, KH, 512], bf, tag="xt", name="xt")
            if j == 0:
                for q in range(4):
                    nc.sync.dma_start(
                        xt_t[:, q * 4:(q + 1) * 4], xt_d[j, :, q * 4:(q + 1) * 4]
                    )
            else:
                nc.sync.dma_start(xt_t[:], xt_d[j])
            as_j = as_pool.tile([128, NFS, 512], bf, tag="asj", name="asj")
            for f in range(NFS):
                wsg_t = wgu_pool.tile([128, KH, 128], bf, tag="wg")
                wsu_t = wgu_pool.tile([128, KH, 128], bf, tag="wu")
                if j == 0 and f == 0:
                    nc.sync.dma_start(wsg_t[:, :8], wsg_d[f, :, :8])
                    nc.sync.dma_start(wsg_t[:, 8:], wsg_d[f, :, 8:])
                    nc.sync.dma_start(wsu_t[:, :8], wsu_d[f, :, :8])
                    nc.sync.dma_start(wsu_t[:, 8:], wsu_d[f, :, 8:])
                else:
                    nc.sync.dma_start(wsg_t[:], wsg_d[f])
                    nc.sync.dma_start(wsu_t[:], wsu_d[f])
                pg = psum_gu.tile([128, 512], f32, tag="pg", name="pg")
                pu = psum_gu.tile([128, 512], f32, tag="pu", name="pu")
                for k in range(KH):
                    nc.tensor.matmul(
                        pg, wsg_t[:, k], xt_t[:, k],
                        start=(k == 0), stop=(k == KH - 1),
                    )
                for k in range(KH):
                    nc.tensor.matmul(
                        pu, wsu_t[:, k], xt_t[:, k],
                        start=(k == 0), stop=(k == KH - 1),
                    )
                sg = sg_pool.tile([128, 512], f32, tag="sg", name="sg")
                nc.scalar.activation(sg, pg, Silu, bias=bias0[:])
                nc.vector.tensor_mul(as_j[:, f], sg, pu)
            if not wsd_ts:
                for h in range(NH):
                    w = wsd_pool.tile(
                        [128, NFS, 512], bf, tag=f"wsd{h}", name=f"wsd{h}"
                    )
                    nc.sync.dma_start(w[:], wsd_d[h])
                    wsd_ts.append(w)
            # down-proj for this T-chunk right away: spreads sout writes
            for tl in range(4):
                for h in range(NH):
                    pd = psum_d.tile([128, 512], f32, tag="pd")
                    for fo in range(NFS):
                        nc.tensor.matmul(
                            pd, as_j[:, fo, tl * 128:(tl + 1) * 128],
                            wsd_ts[h][:, fo],
                            start=(fo == 0), stop=(fo == NFS - 1),
                        )
                    ob = ev_pool.tile([128, 512], bf, tag="ob")
                    nc.any.tensor_copy(out=ob[:], in_=pd[:])
                    nc.sync.dma_start(
                        sout[j * 512 + tl * 128: j * 512 + (tl + 1) * 128,
                             h * 512:(h + 1) * 512],
                        ob[:],
                    )

        # ---- routed experts (2 per core, fp8 DoubleRow) ----
        with (
            tc.tile_pool(name="xgp", bufs=1) as xg_pool,
            tc.tile_pool(name="wdp", bufs=2) as wd_pool,
            tc.tile_pool(name="atp", bufs=1) as a_pool,
        ):
            for e in range(EPC):
                C = caps[e]
                CT = CTs[e]
                cw = _chunks(C)
                nch = C // cw
                xg_t = xg_pool.tile([128, KH, C], fp8, tag=f"xg{e}", name=f"xg{e}")
                nc.sync.dma_start(xg_t[:], xg_ds[e][:])
                aT = a_pool.tile([128, NF, C], fp8, tag=f"aT{e}", name=f"aT{e}")
                for f in range(NF):
                    wg_t = wgu_pool.tile([128, KH, 128], fp8, tag="wg")
                    nc.sync.dma_start(wg_t[:], wg_d[e, f])
                    wu_t = wgu_pool.tile([128, KH, 128], fp8, tag="wu")
                    nc.sync.dma_start(wu_t[:], wu_d[e, f])
                    for j in range(nch):
                        csl = slice(j * cw, (j + 1) * cw)
                        pg = psum_gu.tile([128, 512], f32, tag="pg", name="pg")[:, :cw]
                        pu = psum_gu.tile([128, 512], f32, tag="pu", name="pu")[:, :cw]
                        for k in range(KH2):
                            nc.tensor.matmul(
                                pg, wg_t[:, 2 * k:2 * k + 2],
                                xg_t[:, 2 * k:2 * k + 2, csl],
                                start=(k == 0), stop=(k == KH2 - 1),
                                perf_mode=DR,
                            )
                        for k in range(KH2):
                            nc.tensor.matmul(
                                pu, wu_t[:, 2 * k:2 * k + 2],
                                xg_t[:, 2 * k:2 * k + 2, csl],
                                start=(k == 0), stop=(k == KH2 - 1),
                                perf_mode=DR,
                            )
                        sg = sg_pool.tile([128, 512], f32, tag="sg", name="sg")[:, :cw]
                        # pg holds 32*g -> silu(g); pu holds 8*u, so the
                        # e4m3 store of sg*pu is a_q = e4m3(8*a)
                        nc.scalar.activation(
                            sg, pg, Silu, bias=bias0[:], scale=1.0 / SWG
                        )
                        nc.vector.tensor_mul(aT[:, f, csl], sg, pu)
                for h in range(NH):
                    wd_t = wd_pool.tile([128, NF, 512], fp8, tag="wd")
                    nc.sync.dma_start(wd_t[:], wd_d[e, h])
                    for ct in range(CT):
                        tw = min(128, C - ct * 128)
                        pd = psum_d.tile([128, 512], f32, tag="pd", name="pd")[:tw]
                        for fo in range(NF2):
                            nc.tensor.matmul(
                                pd,
                                aT[:, 2 * fo:2 * fo + 2, ct * 128: ct * 128 + tw],
                                wd_t[:, 2 * fo:2 * fo + 2],
                                start=(fo == 0), stop=False,
                                perf_mode=DR,
                            )
                        nc.tensor.matmul(
                            pd, aT[:, NF - 1, ct * 128: ct * 128 + tw],
                            wd_t[:, NF - 1],
                            start=False, stop=True,
                        )
                        ob = ev_pool.tile([128, 512], bf, tag="ob", name="ob")[:tw]
                        col = sum(CTs[:e]) + ct
                        nc.vector.tensor_scalar_mul(
                            ob[:], pd[:], wts_t[:tw, col:col + 1]
                        )
                        nc.sync.dma_start(
                            rout[off[e] + ct * 128: off[e] + ct * 128 + tw,
                                 h * 512:(h + 1) * 512],
                            ob[:],
                        )

    nc.compile()
    _BUILD_CACHE[key] = nc
    return nc


def kernel(**inputs: np.ndarray) -> np.ndarray:
    global last_exec_time_ns
    from concourse.bass_utils import run_bass_kernel_spmd

    hs = inputs["hidden_states"]
    x = np.ascontiguousarray(hs.reshape(T, H), dtype=np.float32)
    top_idx, top_vals = _routing(x, inputs["gate_weight"])

    # per-expert token lists (ascending token order)
    rows_per_e = []
    for e in range(E):
        rows, kpos = np.nonzero(top_idx == e)
        rows_per_e.append((rows, top_vals[rows, kpos]))
    counts = np.array([len(r) for r, _ in rows_per_e])
    # rank experts by load: ranks 0..7 -> slot 0 of cores 0..7 (big slots),
    # ranks 8..15 -> slot 1 of cores 7..0 (small slots)
    order = np.argsort(-counts, kind="stable")
    slot_expert = np.empty((N_CORES, EPC), np.int64)
    for i in range(N_CORES):
        slot_expert[i, 0] = order[i]
        slot_expert[i, 1] = order[E - 1 - i]
    cap = lambda n: max(128, ((n + 63) // 64) * 64)
    caps = (
        cap(int(counts[slot_expert[:, 0]].max())),
        cap(int(counts[slot_expert[:, 1]].max())),
    )
    CTs = [(C + 127) // 128 for C in caps]
    CTsum = sum(CTs)

    nc = _build(caps)

    xb = x.astype(BF16)
    # xt chunks [NT, 128, KH, 512]: xt[j, p, k, t'] = x[j*512+t', k*128+p]
    xtR = np.ascontiguousarray(xb.reshape(NT, 512, KH, 128).transpose(0, 3, 2, 1))

    w_gate = inputs["w_gate"]
    w_up = inputs["w_up"]
    w_down = inputs["w_down"]
    ws_gate = inputs["ws_gate"].astype(BF16)
    ws_up = inputs["ws_up"].astype(BF16)
    ws_down = inputs["ws_down"].astype(BF16)

    in_maps = []
    for c in range(N_CORES):
        wtsR = np.zeros((128, CTsum), np.float32)
        wgR = np.empty((EPC, NF, 128, KH, 128), E4)
        wuR = np.empty((EPC, NF, 128, KH, 128), E4)
        wdR = np.empty((EPC, NH, 128, NF, 512), E4)
        imap = {}
        for el in range(EPC):
            C = caps[el]
            CT = CTs[el]
            ge = int(slot_expert[c, el])
            rows, wts = rows_per_e[ge]
            n = len(rows)
            xgR = np.zeros((128, KH, C), E4)
            if n:
                # [n, H] -> [128, KH, n], f32 -> e4m3 directly
                xgR[:, :, :n] = (
                    x[rows].astype(E4).reshape(n, KH, 128).transpose(2, 1, 0)
                )
                wcol = np.zeros(CT * 128, np.float32)
                wcol[:n] = wts / WTS_DIV
                base = sum(CTs[:el])
                wtsR[:, base:base + CT] = wcol.reshape(CT, 128).T
            imap[f"xg{el}"] = xgR
            wgR[el] = (
                (SWG * w_gate[ge]).astype(E4)
                .reshape(KH, 128, NF, 128).transpose(2, 1, 0, 3)
            )
            wuR[el] = (
                (SWU * w_up[ge]).astype(E4)
                .reshape(KH, 128, NF, 128).transpose(2, 1, 0, 3)
            )
            wdR[el] = (
                (SD * w_down[ge]).astype(E4)
                .reshape(NF, 128, NH, 512).transpose(2, 1, 0, 3)
            )
        sl = slice(c * FSS, (c + 1) * FSS)
        wsgp = np.zeros((H, FSP), BF16)
        wsgp[:, :FSS] = ws_gate[:, sl]
        wsup = np.zeros((H, FSP), BF16)
        wsup[:, :FSS] = ws_up[:, sl]
        wsdp = np.zeros((FSP, H), BF16)
        wsdp[:FSS] = ws_down[sl]
        imap.update(
            wg=wgR,
            wu=wuR,
            wd=wdR,
            xt=xtR,
            wsg=np.ascontiguousarray(
                wsgp.reshape(KH, 128, NFS, 128).transpose(2, 1, 0, 3)
            ),
            wsu=np.ascontiguousarray(
                wsup.reshape(KH, 128, NFS, 128).transpose(2, 1, 0, 3)
            ),
            wsd=np.ascontiguousarray(
                wsdp.reshape(NFS, 128, NH, 512).transpose(2, 1, 0, 3)
            ),
            wts=wtsR,
        )
        in_maps.append(imap)

    res = run_bass_kernel_spmd(nc, in_maps, core_ids=list(range(N_CORES)))
    last_exec_time_ns = res.exec_time_ns

    out = np.zeros((T, H), np.float32)
    off = [0, caps[0]]
    for c in range(N_CORES):
        r = res.results[c]
        out += r["sout"].astype(np.float32)
        for el in range(EPC):
            rows, _ = rows_per_e[int(slot_expert[c, el])]
            n = len(rows)
            if n:
                # rows are unique within one expert, so fancy-index add is safe
                out[rows] += r["rout"][off[el]: off[el] + n].astype(np.float32)
    return out.reshape(hs.shape).astype(hs.dtype)


# revision 3
# speedup vs baseline: 1.7624x; 1.0791x over previous
"""Trainium2 Bass kernel for a DeepSeek-style MoE block (full-I/O contract).

Strategy (8 NeuronCores):
  - Expert-parallel: E=16 routed experts, 2 per core. Host computes the gate
    (softmax + top-4) in numpy, gathers each expert's tokens, and ships
    transposed token blocks per core. Experts are ranked by token count:
    ranks 0-7 go to slot 0 (capacity C0), ranks 8-15 to slot 1 (C1 <= C0),
    so padding waste tracks the actual load distribution.
  - Routed experts run in fp8 (e4m3) with DoubleRow matmuls (2x PE
    throughput): weights are host-scaled into e4m3 range (w_gate x32,
    w_up x8, w_down x64), activations quantized on the fly; the silu
    applies 1/32 as its input scale and the routing weights fold the
    remaining 1/(8*64) at PSUM eviction. Accumulation stays fp32.
  - Shared expert stays bf16 (it dominates the output norm; fp8 there
    would blow the error budget) and is split 2D: core c computes token
    block c//2 (512 tokens) x F-half c%2 (1408 of Fs=2816). That shape is
    SPMD-uniform, has zero padding, and reuses the routed-expert loop.
  - Host scatter-adds the routed partials and sums the shared partials.
"""

import math
from contextlib import ExitStack

import ml_dtypes
import numpy as np

T = 2048
H = 2048
E = 16
TOP_K = 4
F = 1408
FS = 2816
N_CORES = 8
EPC = E // N_CORES  # experts per core = 2
KH = H // 128  # 16 contraction chunks over H
KH2 = KH // 2  # 8 DoubleRow pairs
NF = F // 128  # 11 F tiles (also the shared F-half tile count)
NF2 = NF // 2  # 5 DoubleRow pairs (+1 single tail chunk)
NH = H // 512  # 4 output H tiles
CS = T // (N_CORES // 2)  # 512-token shared block per core pair

BF16 = ml_dtypes.bfloat16
E4 = ml_dtypes.float8_e4m3

SWG = 32.0  # w_gate fp8 scale
SWU = 8.0   # w_up fp8 scale (also the a_q scale; keeps |a_q| < 100 << 240)
SD = 64.0   # w_down fp8 scale
WTS_DIV = SWU * SD  # folded into routing weights at eviction

_BUILD_CACHE: dict[tuple, object] = {}
last_exec_time_ns = None


def _routing(x: np.ndarray, gate_weight: np.ndarray):
    """Replicates the reference gate: fp32 logits, softmax, top-4 (ties ->
    lower expert index, matching jax.lax.top_k), no renorm."""
    logits = x.astype(np.float32) @ gate_weight.astype(np.float32).T
    z = logits - logits.max(axis=1, keepdims=True)
    p = np.exp(z)
    p /= p.sum(axis=1, keepdims=True)
    top_idx = np.argsort(-p, axis=1, kind="stable")[:, :TOP_K]
    top_vals = np.take_along_axis(p, top_idx, axis=1).astype(np.float32)
    return top_idx, top_vals


def _chunks(C):
    n = max(1, math.ceil(C / 512))
    while C % n:
        n += 1
    return C // n


def _build(caps: tuple):
    """Build + compile the SPMD one-core Bass graph for slot capacities."""
    key = tuple(caps)
    if key in _BUILD_CACHE:
        return _BUILD_CACHE[key]

    import concourse.bass as bass  # noqa: F401
    from concourse import bacc, mybir, tile

    bf = mybir.dt.bfloat16
    f32 = mybir.dt.float32
    fp8 = mybir.dt.float8e4
    DR = mybir.MatmulPerfMode.DoubleRow
    Silu = mybir.ActivationFunctionType.Silu

    CTs = [(C + 127) // 128 for C in caps]
    CTsum = sum(CTs)
    off = [0, caps[0]]  # row offsets into rout

    nc = bacc.Bacc(None, target_bir_lowering=False)

    xg_ds = [
        nc.dram_tensor(f"xg{e}", [128, KH, caps[e]], fp8, kind="ExternalInput")
        for e in range(EPC)
    ]
    wg_d = nc.dram_tensor("wg", [EPC, NF, 128, KH, 128], fp8, kind="ExternalInput")
    wu_d = nc.dram_tensor("wu", [EPC, NF, 128, KH, 128], fp8, kind="ExternalInput")
    wd_d = nc.dram_tensor("wd", [EPC, NH, 128, NF, 512], fp8, kind="ExternalInput")
    xs_d = nc.dram_tensor("xs", [128, KH, CS], bf, kind="ExternalInput")
    wsg_d = nc.dram_tensor("wsg", [NF, 128, KH, 128], bf, kind="ExternalInput")
    wsu_d = nc.dram_tensor("wsu", [NF, 128, KH, 128], bf, kind="ExternalInput")
    wsd_d = nc.dram_tensor("wsd", [NH, 128, NF, 512], bf, kind="ExternalInput")
    wts_d = nc.dram_tensor("wts", [128, CTsum], f32, kind="ExternalInput")
    rout = nc.dram_tensor("rout", [sum(caps), H], bf, kind="ExternalOutput")
    sout = nc.dram_tensor("sout", [CS, H], bf, kind="ExternalOutput")

    with tile.TileContext(nc) as tc, ExitStack() as ctx:
        const = ctx.enter_context(tc.tile_pool(name="const", bufs=1))
        bias0 = const.tile([128, 1], f32)
        nc.vector.memset(bias0[:], 0.0)
        wts_t = const.tile([128, CTsum], f32)
        nc.sync.dma_start(wts_t[:], wts_d[:])

        x_pool = ctx.enter_context(tc.tile_pool(name="xp", bufs=1))
        wgu_pool = ctx.enter_context(tc.tile_pool(name="wgu", bufs=4))
        wd_pool = ctx.enter_context(tc.tile_pool(name="wdp", bufs=2))
        a_pool = ctx.enter_context(tc.tile_pool(name="atp", bufs=1))
        ev_pool = ctx.enter_context(tc.tile_pool(name="evp", bufs=6))
        sg_pool = ctx.enter_context(tc.tile_pool(name="sgp", bufs=4))
        psum_gu = ctx.enter_context(tc.tile_pool(name="pgu", bufs=2, space="PSUM"))
        psum_d = ctx.enter_context(tc.tile_pool(name="pdp", bufs=4, space="PSUM"))

        # HAM warmup: keep the PE busy during the initial DMA wait so the
        # clock-gate is at 8/8 when the first real matmuls arrive
        warm = const.tile([128, 512], bf, name="warm")
        nc.vector.memset(warm[:], 0.0)
        warmout = const.tile([128, 1], f32, name="warmout")
        wpsum = psum_gu.tile([128, 512], f32, tag="pg", name="warmp")
        for _ in range(16):
            nc.tensor.matmul(wpsum, warm[:, :128], warm[:], start=True, stop=True)
        nc.vector.tensor_copy(out=warmout[:], in_=wpsum[:, :1])

        def expert(tag, C, is_fp8, x_d, wg_e, wu_e, wd_e, out_d, out_off, wts_col):
            """One gated-MLP expert: gate/up -> silu*mul -> down.

            fp8 experts run DoubleRow matmuls and apply the routing weight
            (pre-divided by SWU*SD) at eviction; the bf16 shared expert
            evicts with a plain copy."""
            dt = fp8 if is_fp8 else bf
            CT = (C + 127) // 128
            cw = _chunks(C)
            nch = C // cw
            x_t = x_pool.tile([128, KH, C], dt, tag=f"x{tag}", name=f"x{tag}")
            # split the token-chunk columns so the first matmul group only
            # waits on the first chunk's DMA
            for j in range(nch):
                nc.sync.dma_start(
                    x_t[:, :, j * cw:(j + 1) * cw], x_d[:, :, j * cw:(j + 1) * cw]
                )
            aT = a_pool.tile([128, NF, C], dt, tag=f"aT{tag}", name=f"aT{tag}")
            for f in range(NF):
                wg_t = wgu_pool.tile([128, KH, 128], dt, tag=f"wg{is_fp8}")
                nc.sync.dma_start(wg_t[:], wg_e[f])
                wu_t = wgu_pool.tile([128, KH, 128], dt, tag=f"wu{is_fp8}")
                nc.sync.dma_start(wu_t[:], wu_e[f])
                for j in range(nch):
                    csl = slice(j * cw, (j + 1) * cw)
                    pg = psum_gu.tile([128, 512], f32, tag="pg", name="pg")[:, :cw]
                    pu = psum_gu.tile([128, 512], f32, tag="pu", name="pu")[:, :cw]
                    if is_fp8:
                        for k in range(KH2):
                            nc.tensor.matmul(
                                pg, wg_t[:, 2 * k:2 * k + 2],
                                x_t[:, 2 * k:2 * k + 2, csl],
                                start=(k == 0), stop=(k == KH2 - 1), perf_mode=DR,
                            )
                        for k in range(KH2):
                            nc.tensor.matmul(
                                pu, wu_t[:, 2 * k:2 * k + 2],
                                x_t[:, 2 * k:2 * k + 2, csl],
                                start=(k == 0), stop=(k == KH2 - 1), perf_mode=DR,
                            )
                    else:
                        for k in range(KH):
                            nc.tensor.matmul(
                                pg, wg_t[:, k], x_t[:, k, csl],
                                start=(k == 0), stop=(k == KH - 1),
                            )
                        for k in range(KH):
                            nc.tensor.matmul(
                                pu, wu_t[:, k], x_t[:, k, csl],
                                start=(k == 0), stop=(k == KH - 1),
                            )
                    sg = sg_pool.tile([128, 512], f32, tag="sg", name="sg")[:, :cw]
                    # fp8: pg holds 32*g -> silu(g); pu holds 8*u, so the
                    # e4m3 store of sg*pu is a_q = e4m3(8*a)
                    nc.scalar.activation(
                        sg, pg, Silu, bias=bias0[:],
                        scale=(1.0 / SWG) if is_fp8 else 1.0,
                    )
                    nc.vector.tensor_mul(aT[:, f, csl], sg, pu)
            for h in range(NH):
                wd_t = wd_pool.tile([128, NF, 512], dt, tag=f"wd{is_fp8}")
                nc.sync.dma_start(wd_t[:], wd_e[h])
                for ct in range(CT):
                    tw = min(128, C - ct * 128)
                    pd = psum_d.tile([128, 512], f32, tag="pd", name="pd")[:tw]
                    if is_fp8:
                        for fo in range(NF2):
                            nc.tensor.matmul(
                                pd,
                                aT[:, 2 * fo:2 * fo + 2, ct * 128: ct * 128 + tw],
                                wd_t[:, 2 * fo:2 * fo + 2],
                                start=(fo == 0), stop=False, perf_mode=DR,
                            )
                        nc.tensor.matmul(
                            pd, aT[:, NF - 1, ct * 128: ct * 128 + tw],
                            wd_t[:, NF - 1],
                            start=False, stop=True,
                        )
                    else:
                        for fo in range(NF):
                            nc.tensor.matmul(
                                pd, aT[:, fo, ct * 128: ct * 128 + tw],
                                wd_t[:, fo],
                                start=(fo == 0), stop=(fo == NF - 1),
                            )
                    ob = ev_pool.tile([128, 512], bf, tag="ob", name="ob")[:tw]
                    if wts_col is not None:
                        nc.vector.tensor_scalar_mul(
                            ob[:], pd[:], wts_t[:tw, wts_col + ct:wts_col + ct + 1]
                        )
                    else:
                        nc.any.tensor_copy(out=ob[:], in_=pd[:])
                    nc.sync.dma_start(
                        out_d[out_off + ct * 128: out_off + ct * 128 + tw,
                              h * 512:(h + 1) * 512],
                        ob[:],
                    )

        # slot-0 routed expert first: its fp8 token block is the smallest
        # initial DMA, so real matmuls start soonest after warmup
        expert("r0", caps[0], True, xg_ds[0], wg_d[0], wu_d[0], wd_d[0],
               rout, off[0], 0)
        expert("s", CS, False, xs_d, wsg_d, wsu_d, wsd_d, sout, 0, None)
        expert("r1", caps[1], True, xg_ds[1], wg_d[1], wu_d[1], wd_d[1],
               rout, off[1], CTs[0])

    nc.compile()
    _BUILD_CACHE[key] = nc
    return nc


def kernel(**inputs: np.ndarray) -> np.ndarray:
    global last_exec_time_ns
    from concourse.bass_utils import run_bass_kernel_spmd

    hs = inputs["hidden_states"]
    x = np.ascontiguousarray(hs.reshape(T, H), dtype=np.float32)
    top_idx, top_vals = _routing(x, inputs["gate_weight"])

    # per-expert token lists (ascending token order)
    rows_per_e = []
    for e in range(E):
        rows, kpos = np.nonzero(top_idx == e)
        rows_per_e.append((rows, top_vals[rows, kpos]))
    counts = np.array([len(r) for r, _ in rows_per_e])
    # rank experts by load: ranks 0..7 -> slot 0 of cores 0..7 (big slots),
    # ranks 8..15 -> slot 1 of cores 7..0 (small slots)
    order = np.argsort(-counts, kind="stable")
    slot_expert = np.empty((N_CORES, EPC), np.int64)
    for i in range(N_CORES):
        slot_expert[i, 0] = order[i]
        slot_expert[i, 1] = order[E - 1 - i]
    cap = lambda n: max(128, ((n + 63) // 64) * 64)
    caps = (
        cap(int(counts[slot_expert[:, 0]].max())),
        cap(int(counts[slot_expert[:, 1]].max())),
    )
    CTs = [(C + 127) // 128 for C in caps]
    CTsum = sum(CTs)

    nc = _build(caps)

    xb = x.astype(BF16)
    # per-block shared tokens [128, KH, 512]: xs[p, k, t'] = x[b*512+t', k*128+p]
    xsR = [
        np.ascontiguousarray(
            xb[b * CS:(b + 1) * CS].reshape(CS, KH, 128).transpose(2, 1, 0)
        )
        for b in range(N_CORES // 2)
    ]

    w_gate = inputs["w_gate"]
    w_up = inputs["w_up"]
    w_down = inputs["w_down"]
    ws_gate = inputs["ws_gate"].astype(BF16)
    ws_up = inputs["ws_up"].astype(BF16)
    ws_down = inputs["ws_down"].astype(BF16)
    # per-F-half shared weight layouts
    wsgR = [
        np.ascontiguousarray(
            ws_gate[:, half * F:(half + 1) * F]
            .reshape(KH, 128, NF, 128).transpose(2, 1, 0, 3)
        )
        for half in range(2)
    ]
    wsuR = [
        np.ascontiguousarray(
            ws_up[:, half * F:(half + 1) * F]
            .reshape(KH, 128, NF, 128).transpose(2, 1, 0, 3)
        )
        for half in range(2)
    ]
    wsdR = [
        np.ascontiguousarray(
            ws_down[half * F:(half + 1) * F]
            .reshape(NF, 128, NH, 512).transpose(2, 1, 0, 3)
        )
        for half in range(2)
    ]

    in_maps = []
    for c in range(N_CORES):
        wtsR = np.zeros((128, CTsum), np.float32)
        wgR = np.empty((EPC, NF, 128, KH, 128), E4)
        wuR = np.empty((EPC, NF, 128, KH, 128), E4)
        wdR = np.empty((EPC, NH, 128, NF, 512), E4)
        imap = {}
        for el in range(EPC):
            C = caps[el]
            CT = CTs[el]
            ge = int(slot_expert[c, el])
            rows, wts = rows_per_e[ge]
            n = len(rows)
            xgR = np.zeros((128, KH, C), E4)
            if n:
                # [n, H] -> [128, KH, n], f32 -> e4m3 directly
                xgR[:, :, :n] = (
                    x[rows].astype(E4).reshape(n, KH, 128).transpose(2, 1, 0)
                )
                wcol = np.zeros(CT * 128, np.float32)
                wcol[:n] = wts / WTS_DIV
                base = sum(CTs[:el])
                wtsR[:, base:base + CT] = wcol.reshape(CT, 128).T
            imap[f"xg{el}"] = xgR
            wgR[el] = (
                (SWG * w_gate[ge]).astype(E4)
                .reshape(KH, 128, NF, 128).transpose(2, 1, 0, 3)
            )
            wuR[el] = (
                (SWU * w_up[ge]).astype(E4)
                .reshape(KH, 128, NF, 128).transpose(2, 1, 0, 3)
            )
            wdR[el] = (
                (SD * w_down[ge]).astype(E4)
                .reshape(NF, 128, NH, 512).transpose(2, 1, 0, 3)
            )
        imap.update(
            wg=wgR,
            wu=wuR,
            wd=wdR,
            xs=xsR[c // 2],
            wsg=wsgR[c % 2],
            wsu=wsuR[c % 2],
            wsd=wsdR[c % 2],
            wts=wtsR,
        )
        in_maps.append(imap)

    res = run_bass_kernel_spmd(nc, in_maps, core_ids=list(range(N_CORES)))
    last_exec_time_ns = res.exec_time_ns

    out = np.zeros((T, H), np.float32)
    off = [0, caps[0]]
    for c in range(N_CORES):
        r = res.results[c]
        b = c // 2
        out[b * CS:(b + 1) * CS] += r["sout"].astype(np.float32)
        for el in range(EPC):
            rows, _ = rows_per_e[int(slot_expert[c, el])]
            n = len(rows)
            if n:
                # rows are unique within one expert, so fancy-index add is safe
                out[rows] += r["rout"][off[el]: off[el] + n].astype(np.float32)
    return out.reshape(hs.shape).astype(hs.dtype)
